# revision 34
# baseline (speedup 1.0000x reference)
"""Bahdanau attention kernel for Trainium2 (8 NeuronCores, SPMD data-parallel).

Reference computation (per batch b):
    f_proj = features[b] @ W1_w + W1_b            # [T, U]
    h_proj = hidden[b] @ W2_w + W2_b              # [U]
    score  = tanh(f_proj + h_proj) @ V_w + V_b    # [T]
    attn   = softmax(score)                       # [T]
    context[b] = sum_t attn[t] * features[b, t]   # [D]

Sharding: data-parallel over batch (64 batches / 8 cores = 8 per core),
weights replicated.

Per-core dataflow (bf16 matmul operands, fp32 accumulation / biases):
  - F chunks [128, 4(tile), 512(d)] are cast-DMA'd (SWDGE, fp32->bf16)
    straight from HBM; PE-transposes (bf16) produce F^T [128(d), t] for
    the main matmul; bf16 PSUM->SBUF copies run in DVE 2x mode.
  - main matmul computes f_proj TRANSPOSED: [u(part), t(free)] =
    W1_chunk^T @ F^T in bf16, so the (W1_b + h_proj) bias is a
    per-partition scalar that fuses into the ACT Tanh instruction.
  - score is produced directly as COLUMNS: 16 tiny ap=1 matmuls
    (lhsT = tanh block [u,t-block], rhs = V column) contract over
    u-partitions, accumulating the 4 u-chunks in PSUM.  exp runs on the
    [128, 4] column tile (V_b dropped - softmax shift-invariant) with a
    fused accum_out partial row-sum; e lands pre-padded in a zeroed
    [128, 4, 32] buffer so the context matmuls are col-group tiled
    (tile_position), 4 concurrent quadrants, one accumulation group per
    t-tile, folded once per batch with a ones-matmul.
  - score (chunk c) runs at iteration c+2 and context at c+3, with the
    16 F-transposes emitted between the stages, so no PE instruction
    ever waits on tanh/exp/DVE latency.
  - no max-subtraction: |score| <= ||V||_1 ~ 18, safe in fp32.
"""

import sys

for _p in ("/opt/trn_rl_repo", "/opt/pypackages"):
    if _p not in sys.path:
        sys.path.insert(0, _p)

import numpy as np

B, T, D, U = 64, 2048, 512, 512
NCORES = 8
BPC = B // NCORES          # batches per core
PART = 128
DC = D // PART             # 4 contraction chunks
UC = U // PART             # 4 u chunks
TCHUNK = 512               # t columns processed per main-matmul group
TILES_PER_CHUNK = TCHUNK // PART          # 4
NCHUNKS = (BPC * T) // TCHUNK             # 32
CHUNKS_PER_BATCH = T // TCHUNK            # 4

_BUILD_CACHE = {}


def build_bass(tag="v5"):
    """Build + compile the per-core Bass program (same on all cores)."""
    if tag in _BUILD_CACHE:
        return _BUILD_CACHE[tag]

    import concourse.mybir as mybir
    import concourse.tile as tile
    from concourse import bacc
    from concourse.bass import ts
    from concourse.masks import make_identity

    f32 = mybir.dt.float32
    mdt = mybir.dt.float32r   # fp32 (replay) for the hidden-path matmuls
    bdt = mybir.dt.bfloat16   # matmul operand dtype on the hot path
    ACT = mybir.ActivationFunctionType
    AX = mybir.AxisListType

    nc = bacc.Bacc("TRN2", target_bir_lowering=False, debug=False)

    feat = nc.dram_tensor("features", [BPC, T, D], mdt, kind="ExternalInput")
    hid = nc.dram_tensor("hidden", [BPC, D], mdt, kind="ExternalInput")
    w1 = nc.dram_tensor("W1_w", [D, U], f32, kind="ExternalInput")
    b1 = nc.dram_tensor("W1_b", [U], f32, kind="ExternalInput")
    w2 = nc.dram_tensor("W2_w", [D, U], mdt, kind="ExternalInput")
    b2 = nc.dram_tensor("W2_b", [U], f32, kind="ExternalInput")
    vw = nc.dram_tensor("V_w", [U, 1], f32, kind="ExternalInput")
    vb = nc.dram_tensor("V_b", [1], f32, kind="ExternalInput")  # noqa: F841 (shift-invariant under softmax)
    out = nc.dram_tensor("context", [BPC, D], f32, kind="ExternalOutput")

    with tile.TileContext(nc) as tc:
        with (
            tc.tile_pool(name="consts", bufs=1) as consts,
            tc.tile_pool(name="fbig", bufs=7) as fbigp,
            tc.tile_pool(name="ftb", bufs=3) as ftb,
            tc.tile_pool(name="tanh", bufs=3) as tanhp,
            tc.tile_pool(name="small", bufs=3) as small,
            tc.tile_pool(name="outp", bufs=2) as outp,
            tc.tile_pool(name="ps_mm", bufs=2, space="PSUM") as ps_mm,
            tc.tile_pool(name="ps_t", bufs=3, space="PSUM") as ps_t,
            tc.tile_pool(name="ps_s", bufs=1, space="PSUM") as ps_s,
            tc.tile_pool(name="ps_c", bufs=1, space="PSUM") as ps_c,
        ):
            # ---------------- constants / setup ----------------
            ident_f32 = consts.tile([PART, PART], f32)
            make_identity(nc, ident_f32)
            ident_r = consts.tile([PART, PART], mdt)
            nc.vector.tensor_copy(ident_r, ident_f32)
            ident_b = consts.tile([PART, PART], bdt)
            nc.vector.tensor_copy(ident_b, ident_f32)
            ones_f = consts.tile([PART, 1], f32)
            nc.gpsimd.memset(ones_f, 1.0)
            ones_b = consts.tile([PART, 1], bdt)
            nc.vector.tensor_copy(ones_b, ones_f)

            # F chunks: one SWDGE cast-DMA each (fp32 HBM -> bf16 SBUF)
            fb_tiles = {}

            def load(c, split=1):
                if c >= NCHUNKS or c in fb_tiles:
                    return
                b_ = c // CHUNKS_PER_BATCH
                t0 = (c % CHUNKS_PER_BATCH) * TCHUNK
                fb = fbigp.tile(
                    [PART, TILES_PER_CHUNK, D], bdt, tag="F", name=f"fb_{c % 8}"
                )
                src = feat.ap()[b_, t0 : t0 + TCHUNK, :].rearrange(
                    "(f p) d -> p f d", p=PART
                )
                step = TILES_PER_CHUNK // split
                for s in range(split):
                    nc.gpsimd.dma_start(
                        out=fb[:, s * step : (s + 1) * step, :],
                        in_=src[:, s * step : (s + 1) * step, :],
                    )
                fb_tiles[c] = fb

            # chunk 0 split per-tile so its first transpose starts after
            # 256 KB instead of 1 MB; issued ahead of all weight DMAs
            load(0, split=4)
            load(1, split=2)
            load(2)

            # weights: W1/V cast to bf16 in-flight; W2 stays fp32
            w1_sb = consts.tile([PART, DC, U], bdt)
            nc.gpsimd.dma_start(
                out=w1_sb, in_=w1.ap().rearrange("(c p) u -> p c u", p=PART)
            )
            v_sb = consts.tile([PART, UC], bdt)
            nc.gpsimd.dma_start(
                out=v_sb, in_=vw.ap().rearrange("(c p) one -> p (c one)", p=PART)
            )
            w2_sb = consts.tile([PART, DC, U], mdt)
            nc.sync.dma_start(
                out=w2_sb, in_=w2.ap().rearrange("(c p) u -> p c u", p=PART)
            )

            # pre-zeroed e-column buffers [128, 4(tile), 32]: exp writes
            # column 0 each chunk, columns 1..31 stay zero forever, so the
            # context matmuls see a zero-padded M=32 stationary operand.
            # Two buffers, manually alternated by chunk parity.
            vz = consts.tile([PART, TILES_PER_CHUNK * 32], f32)
            nc.gpsimd.memset(vz, 0.0)
            ec_bufs = []
            for i in range(2):
                ec = consts.tile(
                    [PART, TILES_PER_CHUNK, 32], bdt, name=f"ec_{i}"
                )
                nc.vector.tensor_copy(ec, vz.rearrange("p (j c) -> p j c", c=32))
                ec_bufs.append(ec)

            # W1_b + W2_b as per-partition columns [128, uc]
            b1_sb = consts.tile([PART, UC], f32)
            nc.sync.dma_start(out=b1_sb, in_=b1.ap().rearrange("(c p) -> p c", p=PART))
            b2_sb = consts.tile([PART, UC], f32)
            nc.sync.dma_start(out=b2_sb, in_=b2.ap().rearrange("(c p) -> p c", p=PART))
            b12_sb = consts.tile([PART, UC], f32)
            nc.vector.tensor_add(b12_sb, b1_sb, b2_sb)

            # hidden [BPC, D] -> hiddenT [128(d), dc, BPC]
            hid_sb = consts.tile([BPC, D], mdt)
            nc.sync.dma_start(out=hid_sb, in_=hid.ap())
            hidT_sb = consts.tile([PART, DC, BPC], mdt)
            bias_cols = consts.tile([PART, UC, BPC], f32)

            def emit_setup():
                # emitted after chunk 0's transposes so the PE isn't blocked
                # on the weight/hidden DMAs at kernel start
                for dc in range(DC):
                    ps_h = ps_mm.tile([PART, TCHUNK], mdt, tag="mm", name="ps_h")
                    nc.tensor.transpose(
                        ps_h[:, 0:BPC], hid_sb[:, ts(dc, PART)], ident_r[0:BPC, 0:BPC]
                    )
                    nc.vector.tensor_copy(hidT_sb[:, dc, :], ps_h[:, 0:BPC])
                # h_projT[u, b] = sum_dc W2[dc]^T @ hiddenT[dc]  (+W2_b+W1_b)
                for uc in range(UC):
                    ps_h = ps_mm.tile([PART, TCHUNK], f32, tag="mm", name="ps_h2")
                    for dc in range(DC):
                        nc.tensor.matmul(
                            ps_h[:, 0:BPC],
                            w2_sb[:, dc, ts(uc, PART)],
                            hidT_sb[:, dc, :],
                            start=(dc == 0),
                            stop=(dc == DC - 1),
                        )
                    nc.vector.tensor_scalar_add(
                        bias_cols[:, uc, :], ps_h[:, 0:BPC], b12_sb[:, uc : uc + 1]
                    )

            # ---------------- pipeline stages ----------------
            states = {}          # chunk -> in-flight state
            batch_tiles = {}     # batch -> {ps_cpar, s_cols}

            def emit_score(st):
                b, cib = st["b"], st["cib"]
                if cib == 0:
                    batch_tiles[b] = {
                        "ps_cpar": ps_c.tile([PART, D], f32, tag="ctx", name="ps_cpar"),
                        "s_cols": small.tile([PART, CHUNKS_PER_BATCH], f32, tag="ssum", name="s_cols"),
                    }
                # score columns [128(t), 4(tile)]: contract tanh over
                # u-partitions with V columns; 16 tiny ap=1 matmuls
                tanh_sb = st.pop("tanh")
                ps_scol = ps_s.tile([PART, TILES_PER_CHUNK], f32, tag="scol", name="ps_scol")
                for j in range(TILES_PER_CHUNK):
                    for uc in range(UC):
                        nc.tensor.matmul(
                            ps_scol[:, j : j + 1],
                            tanh_sb[:, uc, ts(j, PART)],
                            v_sb[:, uc : uc + 1],
                            start=(uc == 0),
                            stop=(uc == UC - 1),
                            skip_group_check=True,
                        )
                st["ps_scol"] = ps_scol

            def emit_exp(st):
                b, cib = st["b"], st["cib"]
                ec = ec_bufs[st["c"] % 2]
                # e = exp(score) into column 0 of the padded buffer, with
                # fused per-t-partition partial row-sums (V_b dropped:
                # softmax is shift-invariant)
                nc.scalar.activation(
                    ec[:, :, 0:1],
                    st.pop("ps_scol"),
                    ACT.Exp,
                    accum_out=batch_tiles[b]["s_cols"][:, cib : cib + 1],
                )

            def emit_ctx(st):
                b, cib = st["b"], st["cib"]
                bt = batch_tiles[b]
                ps_cpar = bt["ps_cpar"]
                ec = ec_bufs[st["c"] % 2]
                # context partials: 4 col-group-tiled M=32 matmuls, one
                # accumulation group per j, folded once per batch
                fb = st.pop("fb")
                for j in range(TILES_PER_CHUNK):
                    nc.tensor.matmul(
                        ps_cpar[32 * j : 32 * j + 32, :],
                        ec[:, j, :],
                        fb[:, j, :],
                        start=(cib == 0),
                        stop=(cib == CHUNKS_PER_BATCH - 1),
                        tile_position=(0, 32 * j),
                        skip_group_check=True,
                    )
                if cib == CHUNKS_PER_BATCH - 1:
                    cpar_sb = small.tile([PART, D], bdt, tag="cpsb", name="cpar_sb", bufs=2)
                    nc.vector.tensor_copy(cpar_sb, ps_cpar)
                    ps_fin = ps_s.tile([1, D], f32, tag="par", name="ps_fin")
                    nc.tensor.matmul(ps_fin, ones_b, cpar_sb, start=True, stop=True)
                    # sum(e) over the batch: fold s_cols over partitions
                    # (fp32 matmul - tiny), then over the 4 chunk columns
                    ps_x = ps_s.tile([1, CHUNKS_PER_BATCH], f32, tag="scol", name="ps_x")
                    nc.tensor.matmul(ps_x, ones_f, bt["s_cols"], start=True, stop=True)
                    ssum = small.tile([1, 1], f32, tag="ssum1")
                    nc.vector.reduce_sum(ssum, ps_x, axis=AX.X)
                    rec = small.tile([1, 1], f32, tag="rec")
                    nc.vector.reciprocal(rec, ssum)
                    ctx_sb = outp.tile([1, D], f32, tag="ctx_sb")
                    nc.vector.tensor_scalar_mul(ctx_sb, ps_fin, rec)
                    nc.sync.dma_start(out=out.ap()[b : b + 1, :], in_=ctx_sb)
                    del batch_tiles[b]

            s1_state = {}

            def emit_s1(c):
                # PE-transpose chunk c's F tiles (bf16) into F^T layout
                fb = fb_tiles[c]
                ftile_big = ftb.tile([PART, DC, TCHUNK], bdt, tag="FT", name="ftile_big")
                for j in range(TILES_PER_CHUNK):
                    ps_tr = ps_t.tile([PART, TCHUNK], bdt, tag="T", name="ps_tr")
                    for dc in range(DC):
                        nc.tensor.transpose(
                            ps_tr[:, ts(dc, PART)], fb[:, j, ts(dc, PART)], ident_b
                        )
                    nc.vector.tensor_copy(
                        ftile_big[:, :, ts(j, PART)],
                        ps_tr.rearrange("p (c t) -> p c t", c=DC),
                    )
                s1_state[c] = (fb, ftile_big)

            emit_s1(0)

            # stages per iteration `it`:
            #   score-cols(it-2) | F-transposes(it+1) | exp(it-2) |
            #   ctx-quad(it-3) | mains(it)
            for it in range(NCHUNKS + 3):
                c_s = it - 2     # score stage chunk
                c_x = it - 3     # context stage chunk

                if 0 <= c_s < NCHUNKS:
                    emit_score(states[c_s])

                if it + 1 < NCHUNKS:
                    emit_s1(it + 1)
                load(it + 3)
                if it == 0:
                    emit_setup()

                if 0 <= c_s < NCHUNKS:
                    emit_exp(states[c_s])
                if 0 <= c_x < NCHUNKS:
                    emit_ctx(states.pop(c_x))

                if it < NCHUNKS:
                    b = it // CHUNKS_PER_BATCH
                    cib = it % CHUNKS_PER_BATCH
                    fb, ftile_big = s1_state.pop(it)

                    # S2: main matmul + tanh (transposed layout [u, t])
                    tanh_sb = tanhp.tile([PART, UC, TCHUNK], bdt, tag="tanh")
                    for uc in range(UC):
                        ps_f = ps_mm.tile([PART, TCHUNK], f32, tag="mm")
                        for dc in range(DC):
                            nc.tensor.matmul(
                                ps_f,
                                w1_sb[:, dc, ts(uc, PART)],
                                ftile_big[:, dc, :],
                                start=(dc == 0),
                                stop=(dc == DC - 1),
                            )
                        nc.scalar.activation(
                            tanh_sb[:, uc, :],
                            ps_f,
                            ACT.Tanh,
                            bias=bias_cols[:, uc, b : b + 1],
                        )
                    states[it] = {"c": it, "b": b, "cib": cib, "tanh": tanh_sb, "fb": fb}

    nc.compile()
    _BUILD_CACHE[tag] = nc
    return nc


def kernel(**inputs):
    from concourse.bass_utils import run_bass_kernel_spmd

    nc = build_bass()

    feat = np.ascontiguousarray(np.asarray(inputs["features"], dtype=np.float32))
    hid = np.ascontiguousarray(np.asarray(inputs["hidden"], dtype=np.float32))
    shared = {
        k: np.ascontiguousarray(np.asarray(inputs[k], dtype=np.float32))
        for k in ("W1_w", "W1_b", "W2_w", "W2_b", "V_w", "V_b")
    }
    in_maps = []
    for c in range(NCORES):
        m = dict(shared)
        m["features"] = feat[c * BPC : (c + 1) * BPC]
        m["hidden"] = hid[c * BPC : (c + 1) * BPC]
        in_maps.append(m)

    res = run_bass_kernel_spmd(nc, in_maps, list(range(NCORES)))
    return np.concatenate([res.results[c]["context"] for c in range(NCORES)], axis=0)


# revision 35
# speedup vs baseline: 1.0027x; 1.0027x over previous
"""Bahdanau attention kernel for Trainium2 (8 NeuronCores, SPMD data-parallel).

Reference computation (per batch b):
    f_proj = features[b] @ W1_w + W1_b            # [T, U]
    h_proj = hidden[b] @ W2_w + W2_b              # [U]
    score  = tanh(f_proj + h_proj) @ V_w + V_b    # [T]
    attn   = softmax(score)                       # [T]
    context[b] = sum_t attn[t] * features[b, t]   # [D]

Sharding: data-parallel over batch (64 batches / 8 cores = 8 per core),
weights replicated.

Per-core dataflow (bf16 matmul operands, fp32 accumulation / biases):
  - F chunks [128, 4(tile), 512(d)] are cast-DMA'd (SWDGE, fp32->bf16)
    straight from HBM; PE-transposes (bf16) produce F^T [128(d), t] for
    the main matmul; bf16 PSUM->SBUF copies run in DVE 2x mode.
  - main matmul computes f_proj TRANSPOSED: [u(part), t(free)] =
    W1_chunk^T @ F^T in bf16, so the (W1_b + h_proj) bias is a
    per-partition scalar that fuses into the ACT Tanh instruction.
  - score is produced directly as COLUMNS: 16 tiny ap=1 matmuls
    (lhsT = tanh block [u,t-block], rhs = V column) contract over
    u-partitions, accumulating the 4 u-chunks in PSUM.  exp runs on the
    [128, 4] column tile (V_b dropped - softmax shift-invariant) with a
    fused accum_out partial row-sum; e lands pre-padded in a zeroed
    [128, 4, 32] buffer so the context matmuls are col-group tiled
    (tile_position), 4 concurrent quadrants, one accumulation group per
    t-tile, folded once per batch with a ones-matmul.
  - score (chunk c) runs at iteration c+2 and context at c+3, with the
    16 F-transposes emitted between the stages, so no PE instruction
    ever waits on tanh/exp/DVE latency.
  - no max-subtraction: |score| <= ||V||_1 ~ 18, safe in fp32.
"""

import sys

for _p in ("/opt/trn_rl_repo", "/opt/pypackages"):
    if _p not in sys.path:
        sys.path.insert(0, _p)

import numpy as np

B, T, D, U = 64, 2048, 512, 512
NCORES = 8
BPC = B // NCORES          # batches per core
PART = 128
DC = D // PART             # 4 contraction chunks
UC = U // PART             # 4 u chunks
TCHUNK = 512               # t columns processed per main-matmul group
TILES_PER_CHUNK = TCHUNK // PART          # 4
NCHUNKS = (BPC * T) // TCHUNK             # 32
CHUNKS_PER_BATCH = T // TCHUNK            # 4

_BUILD_CACHE = {}


def build_bass(tag="v5"):
    """Build + compile the per-core Bass program (same on all cores)."""
    if tag in _BUILD_CACHE:
        return _BUILD_CACHE[tag]

    import concourse.mybir as mybir
    import concourse.tile as tile
    from concourse import bacc
    from concourse.bass import ts
    from concourse.masks import make_identity

    f32 = mybir.dt.float32
    mdt = mybir.dt.float32r   # fp32 (replay) for the hidden-path matmuls
    bdt = mybir.dt.bfloat16   # matmul operand dtype on the hot path
    ACT = mybir.ActivationFunctionType
    AX = mybir.AxisListType

    nc = bacc.Bacc("TRN2", target_bir_lowering=False, debug=False)

    feat = nc.dram_tensor("features", [BPC, T, D], mdt, kind="ExternalInput")
    hid = nc.dram_tensor("hidden", [BPC, D], mdt, kind="ExternalInput")
    w1 = nc.dram_tensor("W1_w", [D, U], f32, kind="ExternalInput")
    b1 = nc.dram_tensor("W1_b", [U], f32, kind="ExternalInput")
    w2 = nc.dram_tensor("W2_w", [D, U], mdt, kind="ExternalInput")
    b2 = nc.dram_tensor("W2_b", [U], f32, kind="ExternalInput")
    vw = nc.dram_tensor("V_w", [U, 1], f32, kind="ExternalInput")
    vb = nc.dram_tensor("V_b", [1], f32, kind="ExternalInput")  # noqa: F841 (shift-invariant under softmax)
    out = nc.dram_tensor("context", [BPC, D], f32, kind="ExternalOutput")

    with tile.TileContext(nc) as tc:
        with (
            tc.tile_pool(name="consts", bufs=1) as consts,
            tc.tile_pool(name="fbig", bufs=7) as fbigp,
            tc.tile_pool(name="ftb", bufs=3) as ftb,
            tc.tile_pool(name="tanh", bufs=3) as tanhp,
            tc.tile_pool(name="small", bufs=3) as small,
            tc.tile_pool(name="outp", bufs=2) as outp,
            tc.tile_pool(name="ps_mm", bufs=2, space="PSUM") as ps_mm,
            tc.tile_pool(name="ps_t", bufs=3, space="PSUM") as ps_t,
            tc.tile_pool(name="ps_s", bufs=1, space="PSUM") as ps_s,
            tc.tile_pool(name="ps_c", bufs=1, space="PSUM") as ps_c,
        ):
            # ---------------- constants / setup ----------------
            ident_f32 = consts.tile([PART, PART], f32)
            make_identity(nc, ident_f32)
            ident_r = consts.tile([PART, PART], mdt)
            nc.vector.tensor_copy(ident_r, ident_f32)
            ident_b = consts.tile([PART, PART], bdt)
            nc.vector.tensor_copy(ident_b, ident_f32)
            ones_f = consts.tile([PART, 1], f32)
            nc.gpsimd.memset(ones_f, 1.0)
            ones_b = consts.tile([PART, 1], bdt)
            nc.vector.tensor_copy(ones_b, ones_f)

            # F chunks: one SWDGE cast-DMA each (fp32 HBM -> bf16 SBUF)
            fb_tiles = {}

            def load(c, split=1):
                if c >= NCHUNKS or c in fb_tiles:
                    return
                b_ = c // CHUNKS_PER_BATCH
                t0 = (c % CHUNKS_PER_BATCH) * TCHUNK
                fb = fbigp.tile(
                    [PART, TILES_PER_CHUNK, D], bdt, tag="F", name=f"fb_{c % 8}"
                )
                src = feat.ap()[b_, t0 : t0 + TCHUNK, :].rearrange(
                    "(f p) d -> p f d", p=PART
                )
                step = TILES_PER_CHUNK // split
                for s in range(split):
                    nc.gpsimd.dma_start(
                        out=fb[:, s * step : (s + 1) * step, :],
                        in_=src[:, s * step : (s + 1) * step, :],
                    )
                fb_tiles[c] = fb

            # chunk 0 split per-tile so its first transpose starts after
            # 256 KB instead of 1 MB; issued ahead of all weight DMAs
            load(0, split=4)
            load(1, split=2)
            load(2)

            # weights: W1/V cast to bf16 in-flight; W2 stays fp32
            w1_sb = consts.tile([PART, DC, U], bdt)
            nc.gpsimd.dma_start(
                out=w1_sb, in_=w1.ap().rearrange("(c p) u -> p c u", p=PART)
            )
            v_sb = consts.tile([PART, UC], bdt)
            nc.gpsimd.dma_start(
                out=v_sb, in_=vw.ap().rearrange("(c p) one -> p (c one)", p=PART)
            )
            w2_sb = consts.tile([PART, DC, U], mdt)
            nc.sync.dma_start(
                out=w2_sb, in_=w2.ap().rearrange("(c p) u -> p c u", p=PART)
            )

            # pre-zeroed e-column buffers [128, 4(tile), 32]: exp writes
            # column 0 each chunk, columns 1..31 stay zero forever, so the
            # context matmuls see a zero-padded M=32 stationary operand.
            # Two buffers, manually alternated by chunk parity.
            vz = consts.tile([PART, TILES_PER_CHUNK * 32], f32)
            nc.gpsimd.memset(vz, 0.0)
            ec_bufs = []
            for i in range(2):
                ec = consts.tile(
                    [PART, TILES_PER_CHUNK, 32], bdt, name=f"ec_{i}"
                )
                nc.vector.tensor_copy(ec, vz.rearrange("p (j c) -> p j c", c=32))
                ec_bufs.append(ec)

            # W1_b + W2_b as per-partition columns [128, uc]
            b1_sb = consts.tile([PART, UC], f32)
            nc.sync.dma_start(out=b1_sb, in_=b1.ap().rearrange("(c p) -> p c", p=PART))
            b2_sb = consts.tile([PART, UC], f32)
            nc.sync.dma_start(out=b2_sb, in_=b2.ap().rearrange("(c p) -> p c", p=PART))
            b12_sb = consts.tile([PART, UC], f32)
            nc.vector.tensor_add(b12_sb, b1_sb, b2_sb)

            # hidden [BPC, D] -> hiddenT [128(d), dc, BPC]
            hid_sb = consts.tile([BPC, D], mdt)
            nc.sync.dma_start(out=hid_sb, in_=hid.ap())
            hidT_sb = consts.tile([PART, DC, BPC], mdt)
            bias_cols = consts.tile([PART, UC, BPC], f32)

            def emit_setup():
                # emitted after chunk 0's transposes so the PE isn't blocked
                # on the weight/hidden DMAs at kernel start
                for dc in range(DC):
                    ps_h = ps_mm.tile([PART, TCHUNK], mdt, tag="mm", name="ps_h")
                    nc.tensor.transpose(
                        ps_h[:, 0:BPC], hid_sb[:, ts(dc, PART)], ident_r[0:BPC, 0:BPC]
                    )
                    nc.vector.tensor_copy(hidT_sb[:, dc, :], ps_h[:, 0:BPC])
                # h_projT[u, b] = sum_dc W2[dc]^T @ hiddenT[dc]  (+W2_b+W1_b)
                for uc in range(UC):
                    ps_h = ps_mm.tile([PART, TCHUNK], f32, tag="mm", name="ps_h2")
                    for dc in range(DC):
                        nc.tensor.matmul(
                            ps_h[:, 0:BPC],
                            w2_sb[:, dc, ts(uc, PART)],
                            hidT_sb[:, dc, :],
                            start=(dc == 0),
                            stop=(dc == DC - 1),
                        )
                    nc.vector.tensor_scalar_add(
                        bias_cols[:, uc, :], ps_h[:, 0:BPC], b12_sb[:, uc : uc + 1]
                    )

            # ---------------- pipeline stages ----------------
            states = {}          # chunk -> in-flight state
            batch_tiles = {}     # batch -> {ps_cpar, s_cols}

            def emit_score(st):
                b, cib = st["b"], st["cib"]
                if cib == 0:
                    batch_tiles[b] = {
                        "s_cols": small.tile([PART, CHUNKS_PER_BATCH], f32, tag="ssum", name="s_cols"),
                    }
                # score columns [128(t), 4(tile)]: contract tanh over
                # u-partitions with V columns; 16 tiny ap=1 matmuls
                tanh_sb = st.pop("tanh")
                ps_scol = ps_s.tile([PART, TILES_PER_CHUNK], f32, tag="scol", name="ps_scol")
                for j in range(TILES_PER_CHUNK):
                    for uc in range(UC):
                        nc.tensor.matmul(
                            ps_scol[:, j : j + 1],
                            tanh_sb[:, uc, ts(j, PART)],
                            v_sb[:, uc : uc + 1],
                            start=(uc == 0),
                            stop=(uc == UC - 1),
                            skip_group_check=True,
                        )
                st["ps_scol"] = ps_scol

            def emit_exp(st):
                b, cib = st["b"], st["cib"]
                ec = ec_bufs[st["c"] % 2]
                # e = exp(score) into column 0 of the padded buffer, with
                # fused per-t-partition partial row-sums (V_b dropped:
                # softmax is shift-invariant)
                nc.scalar.activation(
                    ec[:, :, 0:1],
                    st.pop("ps_scol"),
                    ACT.Exp,
                    accum_out=batch_tiles[b]["s_cols"][:, cib : cib + 1],
                )

            def emit_ctx(st):
                b, cib = st["b"], st["cib"]
                bt = batch_tiles[b]
                if cib == 0:
                    # acquired at its first writer so the ring-1 WAR
                    # dependency sees the previous batch's fold reads
                    bt["ps_cpar"] = ps_c.tile([PART, D], f32, tag="ctx", name="ps_cpar")
                ps_cpar = bt["ps_cpar"]
                ec = ec_bufs[st["c"] % 2]
                # context partials: 4 col-group-tiled M=32 matmuls, one
                # accumulation group per j, folded once per batch
                fb = st.pop("fb")
                for j in range(TILES_PER_CHUNK):
                    nc.tensor.matmul(
                        ps_cpar[32 * j : 32 * j + 32, :],
                        ec[:, j, :],
                        fb[:, j, :],
                        start=(cib == 0),
                        stop=(cib == CHUNKS_PER_BATCH - 1),
                        tile_position=(0, 32 * j),
                        skip_group_check=True,
                    )
                if cib == CHUNKS_PER_BATCH - 1:
                    cpar_sb = small.tile([PART, D], bdt, tag="cpsb", name="cpar_sb", bufs=2)
                    nc.vector.tensor_copy(cpar_sb, ps_cpar)
                    ps_fin = ps_s.tile([1, D], f32, tag="par", name="ps_fin")
                    nc.tensor.matmul(ps_fin, ones_b, cpar_sb, start=True, stop=True)
                    # sum(e) over the batch: fold s_cols over partitions
                    # (fp32 matmul - tiny), then over the 4 chunk columns
                    ps_x = ps_s.tile([1, CHUNKS_PER_BATCH], f32, tag="scol", name="ps_x")
                    nc.tensor.matmul(ps_x, ones_f, bt["s_cols"], start=True, stop=True)
                    ssum = small.tile([1, 1], f32, tag="ssum1")
                    nc.vector.reduce_sum(ssum, ps_x, axis=AX.X)
                    rec = small.tile([1, 1], f32, tag="rec")
                    nc.vector.reciprocal(rec, ssum)
                    ctx_sb = outp.tile([1, D], f32, tag="ctx_sb")
                    nc.vector.tensor_scalar_mul(ctx_sb, ps_fin, rec)
                    nc.sync.dma_start(out=out.ap()[b : b + 1, :], in_=ctx_sb)
                    del batch_tiles[b]

            s1_state = {}

            def emit_s1(c):
                # PE-transpose chunk c's F tiles (bf16) into F^T layout
                fb = fb_tiles[c]
                ftile_big = ftb.tile([PART, DC, TCHUNK], bdt, tag="FT", name="ftile_big")
                for j in range(TILES_PER_CHUNK):
                    ps_tr = ps_t.tile([PART, TCHUNK], bdt, tag="T", name="ps_tr")
                    for dc in range(DC):
                        nc.tensor.transpose(
                            ps_tr[:, ts(dc, PART)], fb[:, j, ts(dc, PART)], ident_b
                        )
                    nc.vector.tensor_copy(
                        ftile_big[:, :, ts(j, PART)],
                        ps_tr.rearrange("p (c t) -> p c t", c=DC),
                    )
                s1_state[c] = (fb, ftile_big)

            emit_s1(0)

            # stages per iteration `it`:
            #   score-cols(it-2) | F-transposes(it+1) | exp(it-2) |
            #   ctx-quad(it-3) | mains(it)
            for it in range(NCHUNKS + 3):
                c_s = it - 2     # score stage chunk
                c_x = it - 3     # context stage chunk

                if 0 <= c_s < NCHUNKS:
                    emit_score(states[c_s])

                if it + 1 < NCHUNKS:
                    emit_s1(it + 1)
                load(it + 3)
                if it == 0:
                    emit_setup()

                if 0 <= c_s < NCHUNKS:
                    emit_exp(states[c_s])
                if 0 <= c_x < NCHUNKS:
                    emit_ctx(states.pop(c_x))

                if it < NCHUNKS:
                    b = it // CHUNKS_PER_BATCH
                    cib = it % CHUNKS_PER_BATCH
                    fb, ftile_big = s1_state.pop(it)

                    # S2: main matmul + tanh (transposed layout [u, t])
                    tanh_sb = tanhp.tile([PART, UC, TCHUNK], bdt, tag="tanh")
                    for uc in range(UC):
                        ps_f = ps_mm.tile([PART, TCHUNK], f32, tag="mm")
                        for dc in range(DC):
                            nc.tensor.matmul(
                                ps_f,
                                w1_sb[:, dc, ts(uc, PART)],
                                ftile_big[:, dc, :],
                                start=(dc == 0),
                                stop=(dc == DC - 1),
                            )
                        nc.scalar.activation(
                            tanh_sb[:, uc, :],
                            ps_f,
                            ACT.Tanh,
                            bias=bias_cols[:, uc, b : b + 1],
                        )
                    states[it] = {"c": it, "b": b, "cib": cib, "tanh": tanh_sb, "fb": fb}

    nc.compile()
    _BUILD_CACHE[tag] = nc
    return nc


def kernel(**inputs):
    from concourse.bass_utils import run_bass_kernel_spmd

    nc = build_bass()

    feat = np.ascontiguousarray(np.asarray(inputs["features"], dtype=np.float32))
    hid = np.ascontiguousarray(np.asarray(inputs["hidden"], dtype=np.float32))
    shared = {
        k: np.ascontiguousarray(np.asarray(inputs[k], dtype=np.float32))
        for k in ("W1_w", "W1_b", "W2_w", "W2_b", "V_w", "V_b")
    }
    in_maps = []
    for c in range(NCORES):
        m = dict(shared)
        m["features"] = feat[c * BPC : (c + 1) * BPC]
        m["hidden"] = hid[c * BPC : (c + 1) * BPC]
        in_maps.append(m)

    res = run_bass_kernel_spmd(nc, in_maps, list(range(NCORES)))
    return np.concatenate([res.results[c]["context"] for c in range(NCORES)], axis=0)


# revision 37
# speedup vs baseline: 1.0082x; 1.0056x over previous
"""Bahdanau attention kernel for Trainium2 (8 NeuronCores, SPMD data-parallel).

Reference computation (per batch b):
    f_proj = features[b] @ W1_w + W1_b            # [T, U]
    h_proj = hidden[b] @ W2_w + W2_b              # [U]
    score  = tanh(f_proj + h_proj) @ V_w + V_b    # [T]
    attn   = softmax(score)                       # [T]
    context[b] = sum_t attn[t] * features[b, t]   # [D]

Sharding: data-parallel over batch (64 batches / 8 cores = 8 per core),
weights replicated.

Per-core dataflow (bf16 matmul operands, fp32 accumulation / biases):
  - F chunks [128, 4(tile), 512(d)] are cast-DMA'd (SWDGE, fp32->bf16)
    straight from HBM; PE-transposes (bf16) produce F^T [128(d), t] for
    the main matmul; bf16 PSUM->SBUF copies run in DVE 2x mode.
  - main matmul computes f_proj TRANSPOSED: [u(part), t(free)] =
    W1_chunk^T @ F^T in bf16, so the (W1_b + h_proj) bias is a
    per-partition scalar that fuses into the ACT Tanh instruction.
  - score is produced directly as COLUMNS: 16 tiny ap=1 matmuls
    (lhsT = tanh block [u,t-block], rhs = V column) contract over
    u-partitions, accumulating the 4 u-chunks in PSUM.  exp runs on the
    [128, 4] column tile (V_b dropped - softmax shift-invariant) with a
    fused accum_out partial row-sum; e lands pre-padded in a zeroed
    [128, 4, 32] buffer so the context matmuls are col-group tiled
    (tile_position), 4 concurrent quadrants, one accumulation group per
    t-tile, folded once per batch with a ones-matmul.
  - score (chunk c) runs at iteration c+2 and context at c+3, with the
    16 F-transposes emitted between the stages, so no PE instruction
    ever waits on tanh/exp/DVE latency.
  - no max-subtraction: |score| <= ||V||_1 ~ 18, safe in fp32.
"""

import sys

for _p in ("/opt/trn_rl_repo", "/opt/pypackages"):
    if _p not in sys.path:
        sys.path.insert(0, _p)

import numpy as np

B, T, D, U = 64, 2048, 512, 512
NCORES = 8
BPC = B // NCORES          # batches per core
PART = 128
DC = D // PART             # 4 contraction chunks
UC = U // PART             # 4 u chunks
TCHUNK = 512               # t columns processed per main-matmul group
TILES_PER_CHUNK = TCHUNK // PART          # 4
NCHUNKS = (BPC * T) // TCHUNK             # 32
CHUNKS_PER_BATCH = T // TCHUNK            # 4

_BUILD_CACHE = {}


def build_bass(tag="v5"):
    """Build + compile the per-core Bass program (same on all cores)."""
    if tag in _BUILD_CACHE:
        return _BUILD_CACHE[tag]

    import concourse.mybir as mybir
    import concourse.tile as tile
    from concourse import bacc
    from concourse.bass import ts
    from concourse.masks import make_identity

    f32 = mybir.dt.float32
    mdt = mybir.dt.float32r   # fp32 (replay) for the hidden-path matmuls
    bdt = mybir.dt.bfloat16   # matmul operand dtype on the hot path
    ACT = mybir.ActivationFunctionType
    AX = mybir.AxisListType

    nc = bacc.Bacc("TRN2", target_bir_lowering=False, debug=False)

    feat = nc.dram_tensor("features", [BPC, T, D], mdt, kind="ExternalInput")
    hid = nc.dram_tensor("hidden", [BPC, D], mdt, kind="ExternalInput")
    w1 = nc.dram_tensor("W1_w", [D, U], f32, kind="ExternalInput")
    b1 = nc.dram_tensor("W1_b", [U], f32, kind="ExternalInput")
    w2 = nc.dram_tensor("W2_w", [D, U], mdt, kind="ExternalInput")
    b2 = nc.dram_tensor("W2_b", [U], f32, kind="ExternalInput")
    vw = nc.dram_tensor("V_w", [U, 1], f32, kind="ExternalInput")
    vb = nc.dram_tensor("V_b", [1], f32, kind="ExternalInput")  # noqa: F841 (shift-invariant under softmax)
    out = nc.dram_tensor("context", [BPC, D], f32, kind="ExternalOutput")

    with tile.TileContext(nc) as tc:
        with (
            tc.tile_pool(name="consts", bufs=1) as consts,
            tc.tile_pool(name="fbig", bufs=7) as fbigp,
            tc.tile_pool(name="ftb", bufs=3) as ftb,
            tc.tile_pool(name="tanh", bufs=3) as tanhp,
            tc.tile_pool(name="small", bufs=3) as small,
            tc.tile_pool(name="outp", bufs=2) as outp,
            tc.tile_pool(name="ps_mm", bufs=2, space="PSUM") as ps_mm,
            tc.tile_pool(name="ps_t", bufs=3, space="PSUM") as ps_t,
            tc.tile_pool(name="ps_s", bufs=1, space="PSUM") as ps_s,
            tc.tile_pool(name="ps_c", bufs=1, space="PSUM") as ps_c,
        ):
            # ---------------- constants / setup ----------------
            ident_f32 = consts.tile([PART, PART], f32)
            make_identity(nc, ident_f32)
            ident_r = consts.tile([PART, PART], mdt)
            nc.vector.tensor_copy(ident_r, ident_f32)
            ident_b = consts.tile([PART, PART], bdt)
            nc.vector.tensor_copy(ident_b, ident_f32)
            ones_f = consts.tile([PART, 1], f32)
            nc.gpsimd.memset(ones_f, 1.0)
            ones_b = consts.tile([PART, 1], bdt)
            nc.vector.tensor_copy(ones_b, ones_f)

            # F chunks: one SWDGE cast-DMA each (fp32 HBM -> bf16 SBUF)
            fb_tiles = {}

            def load(c, split=1):
                if c >= NCHUNKS or c in fb_tiles:
                    return
                b_ = c // CHUNKS_PER_BATCH
                t0 = (c % CHUNKS_PER_BATCH) * TCHUNK
                fb = fbigp.tile(
                    [PART, TILES_PER_CHUNK, D], bdt, tag="F", name=f"fb_{c % 8}"
                )
                src = feat.ap()[b_, t0 : t0 + TCHUNK, :].rearrange(
                    "(f p) d -> p f d", p=PART
                )
                step = TILES_PER_CHUNK // split
                for s in range(split):
                    nc.gpsimd.dma_start(
                        out=fb[:, s * step : (s + 1) * step, :],
                        in_=src[:, s * step : (s + 1) * step, :],
                    )
                fb_tiles[c] = fb

            # chunk 0 split per-tile so its first transpose starts after
            # 256 KB instead of 1 MB; issued ahead of all weight DMAs
            load(0, split=4)
            load(1, split=2)
            load(2)

            # weights: W1/V cast to bf16 in-flight; W2 stays fp32
            w1_sb = consts.tile([PART, DC, U], bdt)
            nc.gpsimd.dma_start(
                out=w1_sb, in_=w1.ap().rearrange("(c p) u -> p c u", p=PART)
            )
            v_sb = consts.tile([PART, UC], bdt)
            nc.gpsimd.dma_start(
                out=v_sb, in_=vw.ap().rearrange("(c p) one -> p (c one)", p=PART)
            )
            w2_sb = consts.tile([PART, DC, U], mdt)
            nc.sync.dma_start(
                out=w2_sb, in_=w2.ap().rearrange("(c p) u -> p c u", p=PART)
            )

            # pre-zeroed e-column buffers [128, 4(tile), 32]: exp writes
            # column 0 each chunk, columns 1..31 stay zero forever, so the
            # context matmuls see a zero-padded M=32 stationary operand.
            # Two buffers, manually alternated by chunk parity.
            vz = consts.tile([PART, TILES_PER_CHUNK * 32], f32)
            nc.gpsimd.memset(vz, 0.0)
            ec_bufs = []
            for i in range(2):
                ec = consts.tile(
                    [PART, TILES_PER_CHUNK, 32], bdt, name=f"ec_{i}"
                )
                nc.vector.tensor_copy(ec, vz.rearrange("p (j c) -> p j c", c=32))
                ec_bufs.append(ec)

            # W1_b + W2_b as per-partition columns [128, uc]
            b1_sb = consts.tile([PART, UC], f32)
            nc.sync.dma_start(out=b1_sb, in_=b1.ap().rearrange("(c p) -> p c", p=PART))
            b2_sb = consts.tile([PART, UC], f32)
            nc.sync.dma_start(out=b2_sb, in_=b2.ap().rearrange("(c p) -> p c", p=PART))
            b12_sb = consts.tile([PART, UC], f32)
            nc.vector.tensor_add(b12_sb, b1_sb, b2_sb)

            # hidden [BPC, D] -> hiddenT [128(d), dc, BPC]
            hid_sb = consts.tile([BPC, D], mdt)
            nc.sync.dma_start(out=hid_sb, in_=hid.ap())
            hidT_sb = consts.tile([PART, DC, BPC], mdt)
            bias_cols = consts.tile([PART, UC, BPC], f32)

            def emit_setup():
                # emitted after chunk 0's transposes so the PE isn't blocked
                # on the weight/hidden DMAs at kernel start
                for dc in range(DC):
                    ps_h = ps_mm.tile([PART, TCHUNK], mdt, tag="mm", name="ps_h")
                    nc.tensor.transpose(
                        ps_h[:, 0:BPC], hid_sb[:, ts(dc, PART)], ident_r[0:BPC, 0:BPC]
                    )
                    nc.vector.tensor_copy(hidT_sb[:, dc, :], ps_h[:, 0:BPC])
                # h_projT[u, b] = sum_dc W2[dc]^T @ hiddenT[dc]  (+W2_b+W1_b)
                for uc in range(UC):
                    ps_h = ps_mm.tile([PART, TCHUNK], f32, tag="mm", name="ps_h2")
                    for dc in range(DC):
                        nc.tensor.matmul(
                            ps_h[:, 0:BPC],
                            w2_sb[:, dc, ts(uc, PART)],
                            hidT_sb[:, dc, :],
                            start=(dc == 0),
                            stop=(dc == DC - 1),
                        )
                    nc.vector.tensor_scalar_add(
                        bias_cols[:, uc, :], ps_h[:, 0:BPC], b12_sb[:, uc : uc + 1]
                    )

            # ---------------- pipeline stages ----------------
            states = {}          # chunk -> in-flight state
            batch_tiles = {}     # batch -> {ps_cpar, s_cols}

            def emit_score(st):
                b, cib = st["b"], st["cib"]
                if cib == 0:
                    batch_tiles[b] = {
                        "s_cols": small.tile([PART, CHUNKS_PER_BATCH], f32, tag="ssum", name="s_cols"),
                    }
                # score columns [128(t), 4(tile)]: contract tanh over
                # u-partitions with V columns; 16 tiny ap=1 matmuls
                tanh_sb = st.pop("tanh")
                ps_scol = ps_s.tile([PART, TILES_PER_CHUNK], f32, tag="scol", name="ps_scol")
                for j in range(TILES_PER_CHUNK):
                    for uc in range(UC):
                        nc.tensor.matmul(
                            ps_scol[:, j : j + 1],
                            tanh_sb[:, uc, ts(j, PART)],
                            v_sb[:, uc : uc + 1],
                            start=(uc == 0),
                            stop=(uc == UC - 1),
                            skip_group_check=True,
                        )
                st["ps_scol"] = ps_scol

            def emit_exp(st):
                b, cib = st["b"], st["cib"]
                ec = ec_bufs[st["c"] % 2]
                # e = exp(score) into column 0 of the padded buffer, with
                # fused per-t-partition partial row-sums (V_b dropped:
                # softmax is shift-invariant)
                nc.scalar.activation(
                    ec[:, :, 0:1],
                    st.pop("ps_scol"),
                    ACT.Exp,
                    accum_out=batch_tiles[b]["s_cols"][:, cib : cib + 1],
                )

            def emit_ctx(st):
                b, cib = st["b"], st["cib"]
                bt = batch_tiles[b]
                if cib == 0:
                    # acquired at its first writer so the ring-1 WAR
                    # dependency sees the previous batch's fold reads
                    bt["ps_cpar"] = ps_c.tile([PART, D], f32, tag="ctx", name="ps_cpar")
                ps_cpar = bt["ps_cpar"]
                ec = ec_bufs[st["c"] % 2]
                # context partials: 4 col-group-tiled M=32 matmuls, one
                # accumulation group per j, folded once per batch
                fb = st.pop("fb")
                for j in range(TILES_PER_CHUNK):
                    nc.tensor.matmul(
                        ps_cpar[32 * j : 32 * j + 32, :],
                        ec[:, j, :],
                        fb[:, j, :],
                        start=(cib == 0),
                        stop=(cib == CHUNKS_PER_BATCH - 1),
                        tile_position=(0, 32 * j),
                        skip_group_check=True,
                    )
                if cib == CHUNKS_PER_BATCH - 1:
                    cpar_sb = small.tile([PART, D], bdt, tag="cpsb", name="cpar_sb", bufs=2)
                    nc.vector.tensor_copy(cpar_sb, ps_cpar)
                    ps_fin = ps_s.tile([1, D], f32, tag="par", name="ps_fin")
                    nc.tensor.matmul(ps_fin, ones_b, cpar_sb, start=True, stop=True)
                    # sum(e) over the batch: fold s_cols over partitions
                    # (fp32 matmul - tiny), then over the 4 chunk columns
                    ps_x = ps_s.tile([1, CHUNKS_PER_BATCH], f32, tag="scol", name="ps_x")
                    nc.tensor.matmul(ps_x, ones_f, bt["s_cols"], start=True, stop=True)
                    ssum = small.tile([1, 1], f32, tag="ssum1")
                    nc.vector.reduce_sum(ssum, ps_x, axis=AX.X)
                    rec = small.tile([1, 1], f32, tag="rec")
                    nc.vector.reciprocal(rec, ssum)
                    ctx_sb = outp.tile([1, D], f32, tag="ctx_sb")
                    nc.vector.tensor_scalar_mul(ctx_sb, ps_fin, rec)
                    nc.sync.dma_start(out=out.ap()[b : b + 1, :], in_=ctx_sb)
                    del batch_tiles[b]

            s1_state = {}

            def emit_s1(c):
                # PE-transpose chunk c's F tiles (bf16) into F^T layout
                fb = fb_tiles[c]
                ftile_big = ftb.tile([PART, DC, TCHUNK], bdt, tag="FT", name="ftile_big")
                for j in range(TILES_PER_CHUNK):
                    ps_tr = ps_t.tile([PART, TCHUNK], bdt, tag="T", name="ps_tr")
                    for dc in range(DC):
                        nc.tensor.transpose(
                            ps_tr[:, ts(dc, PART)], fb[:, j, ts(dc, PART)], ident_b
                        )
                    nc.vector.tensor_copy(
                        ftile_big[:, :, ts(j, PART)],
                        ps_tr.rearrange("p (c t) -> p c t", c=DC),
                    )
                s1_state[c] = (fb, ftile_big)

            emit_s1(0)

            # stages per iteration `it`:
            #   score-cols(it-2) | F-transposes(it+1) | exp(it-2) |
            #   ctx-quad(it-3) | mains(it)
            for it in range(NCHUNKS + 3):
                c_s = it - 2     # score stage chunk
                c_x = it - 3     # context stage chunk

                if 0 <= c_s < NCHUNKS:
                    emit_score(states[c_s])

                if it + 1 < NCHUNKS:
                    emit_s1(it + 1)
                load(it + 3)
                if it == 0:
                    emit_setup()

                if 0 <= c_s < NCHUNKS:
                    emit_exp(states[c_s])
                if 0 <= c_x < NCHUNKS:
                    emit_ctx(states.pop(c_x))

                if it < NCHUNKS:
                    b = it // CHUNKS_PER_BATCH
                    cib = it % CHUNKS_PER_BATCH
                    fb, ftile_big = s1_state.pop(it)

                    # S2: main matmul + tanh (transposed layout [u, t])
                    tanh_sb = tanhp.tile([PART, UC, TCHUNK], bdt, tag="tanh")
                    for uc in range(UC):
                        ps_f = ps_mm.tile([PART, TCHUNK], f32, tag="mm")
                        for dc in range(DC):
                            nc.tensor.matmul(
                                ps_f,
                                w1_sb[:, dc, ts(uc, PART)],
                                ftile_big[:, dc, :],
                                start=(dc == 0),
                                stop=(dc == DC - 1),
                            )
                        nc.scalar.activation(
                            tanh_sb[:, uc, :],
                            ps_f,
                            ACT.Tanh,
                            bias=bias_cols[:, uc, b : b + 1],
                        )
                    states[it] = {"c": it, "b": b, "cib": cib, "tanh": tanh_sb, "fb": fb}

    nc.compile()
    _BUILD_CACHE[tag] = nc
    return nc


def kernel(**inputs):
    from concourse.bass_utils import run_bass_kernel_spmd

    nc = build_bass()

    feat = np.ascontiguousarray(np.asarray(inputs["features"], dtype=np.float32))
    hid = np.ascontiguousarray(np.asarray(inputs["hidden"], dtype=np.float32))
    shared = {
        k: np.ascontiguousarray(np.asarray(inputs[k], dtype=np.float32))
        for k in ("W1_w", "W1_b", "W2_w", "W2_b", "V_w", "V_b")
    }
    in_maps = []
    for c in range(NCORES):
        m = dict(shared)
        m["features"] = feat[c * BPC : (c + 1) * BPC]
        m["hidden"] = hid[c * BPC : (c + 1) * BPC]
        in_maps.append(m)

    res = run_bass_kernel_spmd(nc, in_maps, list(range(NCORES)))
    return np.concatenate([res.results[c]["context"] for c in range(NCORES)], axis=0)
